# revision 2
# baseline (speedup 1.0000x reference)
"""Trainium2 Bass kernel for nn_ChebychevInput.

out[b,o,s] = sum_{i,p} (MAG*coef[o,i,p]) * cos(p*arccos(x[b,i,s])),  p = 0..256

Device pipeline per core (s-shard of 16384 samples, both batches):
  theta stage (flat [96,1024], row = 48b + 16i + sc):
      theta = pi/2 - arctan(x/sqrt(1-x^2))
      th16  = int16(theta * 2^17/(2pi))          # angle in 2^16 half-units
  per (b, sc-chunk of 1024):
      DMA 3 theta rows -> tmp[1, 3072] i16; GPSIMD partition_broadcast
        -> th_bc[128, 3072] i16 (one 1024-col section per input i)
      DVE x2: y32[:, par*3072:+3072] = int32(th_bc * (p/2) + 16384)
        per-partition p/2 from pc[:, par]; k-tile kt=par*3+i covers
        p=par*128+r+1 (6 k-tiles x 128 rows = p 1..256, i-pure)
      ACT one Sin over low halfwords of y32: tm[128, 6144] f16 = cos(p*theta)
      PE per (m, half): 6 accumulating matmuls [128k,128o]x[128k,512s] into a
        single-bank [128,512] PSUM tile (8 tiles rotating over all 8 banks --
        finer-grained than 2-bank tiles, frees banks earlier for the next
        chunk's matmuls)
      evac + p=0 term per (m, half): out_sb col-half = psum + bias[:, m]
        (m=0 halves on DVE, m=1 halves on ACT)
      DMA out_sb -> out[b, m*128:+128, sc*1024:+1024]  (m=0 sync, m=1 scalar)
"""
import sys

sys.path.insert(0, "/opt/trn_rl_repo")

import numpy as np

BATCH = 2
INPUT_DIM = 3
N_SAMPLES = 131072
OUTPUT_DIM = 256
POLY_DEGREE = 256  # p = 0..256
N_CORES = 8
S_SHARD = N_SAMPLES // N_CORES  # 16384
SC = 1024                       # sample chunk
NSC = S_SHARD // SC             # 16
NKT = 6                         # k-tiles of 128 rows: p = 1..256, i = kt//2
WEIGHT_MAGNITUDE = float(np.sqrt(6.0 / (INPUT_DIM * (POLY_DEGREE + 1))))
TWO16 = 65536.0

_compiled = {}


def _build(loop_n=1):
    import concourse.tile as tile
    from concourse import bacc, mybir

    F32 = mybir.dt.float32
    F16 = mybir.dt.float16
    I32 = mybir.dt.int32
    I16 = mybir.dt.int16
    AF = mybir.ActivationFunctionType
    ALU = mybir.AluOpType

    nc = bacc.Bacc("TRN2", target_bir_lowering=False, debug=False)
    x_d = nc.dram_tensor("x", [BATCH, INPUT_DIM, S_SHARD], F32, kind="ExternalInput")
    w_d = nc.dram_tensor("w", [128, NKT * OUTPUT_DIM], F16, kind="ExternalInput")
    pc_d = nc.dram_tensor("pc", [128, 2], F32, kind="ExternalInput")
    bias_d = nc.dram_tensor("bias", [128, 2], F32, kind="ExternalInput")
    out_d = nc.dram_tensor("out", [BATCH, OUTPUT_DIM, S_SHARD], F32, kind="ExternalOutput")

    with tile.TileContext(nc) as tc:
        with (
            tc.tile_pool(name="const", bufs=1) as constp,
            tc.tile_pool(name="theta", bufs=1) as thp,
            tc.tile_pool(name="tmp", bufs=3) as tmpp,
            tc.tile_pool(name="bcast", bufs=3) as bcp,
            tc.tile_pool(name="yint", bufs=2) as yp,
            tc.tile_pool(name="tmat", bufs=3) as tp,
            tc.tile_pool(name="outs", bufs=4) as op,
            tc.tile_pool(name="psum", bufs=8, space="PSUM") as pp,
        ):
            w_t = constp.tile([128, NKT * OUTPUT_DIM], F16)
            nc.sync.dma_start(w_t[:], w_d[:])
            pc_t = constp.tile([128, 2], F32)
            nc.sync.dma_start(pc_t[:], pc_d[:])
            bias_t = constp.tile([128, 2], F32)
            nc.sync.dma_start(bias_t[:], bias_d[:])

            def body():
                # ---- theta stage: flat [96, 1024]; row = 48*b + 16*i + sc
                xt = thp.tile([96, 1024], F32)
                nc.sync.dma_start(xt[:], x_d[:].rearrange("b i (u c) -> (b i u) c", c=1024))
                sq = thp.tile([96, 1024], F32)
                nc.scalar.activation(sq[:], xt[:], AF.Square)
                r2 = thp.tile([96, 1024], F32)
                nc.scalar.activation(r2[:], sq[:], AF.Sqrt, bias=1.0, scale=-1.0)
                inv = thp.tile([96, 1024], F32)
                nc.vector.reciprocal(inv[:], r2[:])
                q = thp.tile([96, 1024], F32)
                nc.vector.tensor_mul(q[:], xt[:], inv[:])
                asn = thp.tile([96, 1024], F32)
                nc.scalar.activation(asn[:], q[:], AF.Arctan)
                # th16 = int16((pi/2 - asn) * 2^17/(2pi)) = int16(32768 - asn*2^17/2pi)
                th16 = thp.tile([96, 1024], I16)
                nc.vector.tensor_scalar(
                    th16[:], asn[:], float(-2.0 * TWO16 / (2 * np.pi)), 32768.0,
                    ALU.mult, ALU.add,
                )
                # repack to one row per (b, chunk): [32, 3*1024] i16, cols =
                # i-sections; 6 plain partition-slice DMAs
                th32 = thp.tile([32, INPUT_DIM * SC], I16)
                for b in range(BATCH):
                    for i in range(INPUT_DIM):
                        nc.sync.dma_start(
                            th32[16 * b:16 * (b + 1), i * SC:(i + 1) * SC],
                            th16[48 * b + 16 * i: 48 * b + 16 * i + 16, :])

                # ---- main loops, software-pipelined: chunk n computes while
                # chunk n-1 evacuates PSUM and DMAs out (keeps each engine's
                # FIFO stream stall-free)
                def evac_and_store(prev):
                    pb, psc, pps = prev
                    for m in range(2):
                        ob = op.tile([128, SC], F32)
                        for half in range(2):
                            sl = slice(half * 512, (half + 1) * 512)
                            if m == 0:
                                nc.vector.tensor_scalar(
                                    ob[:, sl], pps[m][half][:],
                                    bias_t[:, m:m + 1], None, ALU.add)
                            else:
                                nc.scalar.activation(
                                    ob[:, sl], pps[m][half][:], AF.Identity,
                                    bias=bias_t[:, m:m + 1])
                        dst = out_d[pb, m * 128:(m + 1) * 128,
                                    psc * SC:(psc + 1) * SC]
                        if m == 0:
                            nc.sync.dma_start(dst, ob[:])
                        else:
                            nc.scalar.dma_start(dst, ob[:])

                prev = None
                for b in range(BATCH):
                    for sc in range(NSC):
                        row = 16 * b + sc
                        # partition_broadcast needs a partition-0 source
                        tmp = tmpp.tile([1, INPUT_DIM * SC], I16)
                        nc.sync.dma_start(tmp[:], th32[row:row + 1, :])
                        # broadcast viewed as int32 pairs: gpsimd cost is
                        # per-column, so 1536 i32 cols beat 3072 i16 cols 2x
                        th_bc = bcp.tile([128, INPUT_DIM * SC // 2], I32)
                        nc.gpsimd.partition_broadcast(
                            th_bc[:], tmp[0:1, :].bitcast(I32))
                        th_bc16 = th_bc[:].bitcast(I16)

                        # k-tile order kt = parity*3 + i; p = parity*128+r+1,
                        # so one tensor_scalar spans all 3 i-sections per parity
                        y32 = yp.tile([128, NKT * SC], I32)
                        for par in range(2):
                            nc.vector.tensor_scalar(
                                y32[:, par * 3 * SC:(par + 1) * 3 * SC],
                                th_bc16,
                                pc_t[:, par:par + 1], 0.25 * TWO16, ALU.mult, ALU.add,
                            )
                        tm = tp.tile([128, NKT * SC], F16)
                        yv = y32[:].bitcast(I16).rearrange(
                            "p (n two) -> p n two", two=2)[:, :, 0]
                        nc.scalar.activation(tm[:], yv, AF.Sin,
                                             scale=float(2 * np.pi / TWO16))

                        if prev is not None:
                            evac_and_store(prev)

                        pss = []
                        for m in range(2):
                            phs = []
                            for half in range(2):
                                ps = pp.tile([128, 512], F32)
                                for kt in range(NKT):
                                    nc.tensor.matmul(
                                        ps[:],
                                        w_t[:, kt * OUTPUT_DIM + m * 128: kt * OUTPUT_DIM + m * 128 + 128],
                                        tm[:, kt * SC + half * 512: kt * SC + half * 512 + 512],
                                        start=(kt == 0), stop=(kt == NKT - 1),
                                    )
                                phs.append(ps)
                            pss.append(phs)
                        prev = (b, sc, pss)
                evac_and_store(prev)

            if loop_n == 1:
                body()
            else:
                with tc.For_i(0, loop_n, 1):
                    body()
    nc.compile()
    return nc


def _host_prep(coefficients):
    w = (np.asarray(coefficients, dtype=np.float64) * WEIGHT_MAGNITUDE).astype(np.float32)
    # k-tile order kt = parity*3 + i:  wk[r, kt*256+o] = w[o, kt%3, (kt//3)*128+r+1]
    wk = np.empty((128, NKT * OUTPUT_DIM), np.float32)
    for kt in range(NKT):
        i = kt % 3
        p0 = (kt // 3) * 128 + 1
        wk[:, kt * OUTPUT_DIM:(kt + 1) * OUTPUT_DIM] = w[:, i, p0:p0 + 128].T
    r = np.arange(128)
    pc = np.empty((128, 2), np.float32)
    for par in range(2):
        pc[:, par] = (par * 128 + r + 1) * 0.5
    # bias[o', m] = sum_i w[m*128+o', i, 0]
    bias = np.ascontiguousarray(w[:, :, 0].sum(axis=1).reshape(2, 128).T.astype(np.float32))
    return wk.astype(np.float16), pc, bias


def _get_nc(loop_n=1):
    key = ("nc", loop_n)
    if key not in _compiled:
        _compiled[key] = _build(loop_n)
    return _compiled[key]


def _build_callable(nc, n_cores=N_CORES):
    """jit(shard_map(bass_exec)) over the first n_cores devices, mirroring
    run_bass_via_pjrt's lowering; inputs must be device_put with the
    returned sharding (axis 0 = per-core concat)."""
    import jax
    from jax.sharding import Mesh, PartitionSpec, NamedSharding
    from jax.experimental.shard_map import shard_map
    from concourse import mybir
    from concourse.bass2jax import (
        _bass_exec_p, install_neuronx_cc_hook, partition_id_tensor)

    install_neuronx_cc_hook()
    partition_name = nc.partition_id_tensor.name if nc.partition_id_tensor else None

    in_names, out_names, out_avals = [], [], []
    for alloc in nc.m.functions[0].allocations:
        if not isinstance(alloc, mybir.MemoryLocationSet):
            continue
        name = alloc.memorylocations[0].name
        if alloc.kind == "ExternalInput":
            if name != partition_name:
                in_names.append(name)
        elif alloc.kind == "ExternalOutput":
            out_names.append(name)
            out_avals.append(jax.core.ShapedArray(
                tuple(alloc.tensor_shape), mybir.dt.np(alloc.dtype)))
    n_params = len(in_names)
    n_outs = len(out_names)
    all_in_names = in_names + out_names
    if partition_name is not None:
        all_in_names.append(partition_name)

    def _body(*args):
        operands = list(args)
        if partition_name is not None:
            operands.append(partition_id_tensor())
        outs = _bass_exec_p.bind(
            *operands,
            out_avals=tuple(out_avals),
            in_names=tuple(all_in_names),
            out_names=tuple(out_names),
            lowering_input_output_aliases=(),
            sim_require_finite=True,
            sim_require_nnan=True,
            nc=nc,
        )
        return tuple(outs)

    devices = jax.devices()[:n_cores]
    mesh = Mesh(np.asarray(devices), ("core",))
    fn = jax.jit(shard_map(
        _body, mesh=mesh,
        in_specs=(PartitionSpec("core"),) * (n_params + n_outs),
        out_specs=(PartitionSpec("core"),) * n_outs, check_rep=False))
    return fn, NamedSharding(mesh, PartitionSpec("core")), in_names, out_avals


def _prep_globals(x, coefficients):
    """Per-core inputs concatenated along axis 0 (core-major)."""
    wk, pc, bias = _host_prep(coefficients)
    xg = np.ascontiguousarray(
        np.asarray(x, dtype=np.float32).reshape(BATCH, INPUT_DIM, N_CORES, S_SHARD)
        .transpose(2, 0, 1, 3).reshape(N_CORES * BATCH, INPUT_DIM, S_SHARD))
    wg = np.tile(wk, (N_CORES, 1))
    pcg = np.tile(pc, (N_CORES, 1))
    biasg = np.tile(bias, (N_CORES, 1))
    return {"x": xg, "w": wg, "pc": pcg, "bias": biasg}


def kernel(x, coefficients):
    from concourse import bass2jax

    nc = _get_nc()
    wk, pc, bias = _host_prep(coefficients)
    x = np.asarray(x, dtype=np.float32)
    in_maps = [
        {"x": np.ascontiguousarray(x[:, :, c * S_SHARD:(c + 1) * S_SHARD]),
         "w": wk, "pc": pc, "bias": bias}
        for c in range(N_CORES)
    ]
    results = bass2jax.run_bass_via_pjrt(nc, in_maps, n_cores=N_CORES)
    out = np.concatenate([results[c]["out"] for c in range(N_CORES)], axis=2)
    return np.ascontiguousarray(out.astype(np.float32))



# revision 3
# speedup vs baseline: 1.0085x; 1.0085x over previous
"""Trainium2 Bass kernel for nn_ChebychevInput.

out[b,o,s] = sum_{i,p} (MAG*coef[o,i,p]) * cos(p*arccos(x[b,i,s])),  p = 0..256

Device pipeline per core (s-shard of 16384 samples, both batches):
  theta stage (flat [96,1024], row = 48b + 16i + sc):
      theta = pi/2 - arctan(x/sqrt(1-x^2))
      th16  = int16(theta * 2^17/(2pi))          # angle in 2^16 half-units
  per (b, sc-chunk of 1024):
      DMA 3 theta rows -> tmp[1, 3072] i16; GPSIMD partition_broadcast
        -> th_bc[128, 3072] i16 (one 1024-col section per input i)
      DVE x2: y32[:, par*3072:+3072] = int32(th_bc * (p/2) + 16384)
        per-partition p/2 from pc[:, par]; k-tile kt=par*3+i covers
        p=par*128+r+1 (6 k-tiles x 128 rows = p 1..256, i-pure)
      ACT one Sin over low halfwords of y32: tm[128, 6144] f16 = cos(p*theta)
      PE per (m, half): 6 accumulating matmuls [128k,128o]x[128k,512s] into a
        single-bank [128,512] PSUM tile (8 tiles rotating over all 8 banks --
        finer-grained than 2-bank tiles, frees banks earlier for the next
        chunk's matmuls)
      evac + p=0 term per (m, half): out_sb col-half = psum + bias[:, m]
        (m=0 halves on DVE, m=1 halves on ACT)
      DMA out_sb -> out[b, m*128:+128, sc*1024:+1024]  (m=0 sync, m=1 scalar)
"""
import sys

sys.path.insert(0, "/opt/trn_rl_repo")

import numpy as np

BATCH = 2
INPUT_DIM = 3
N_SAMPLES = 131072
OUTPUT_DIM = 256
POLY_DEGREE = 256  # p = 0..256
N_CORES = 8
S_SHARD = N_SAMPLES // N_CORES  # 16384
SC = 1024                       # sample chunk
NSC = S_SHARD // SC             # 16
NKT = 6                         # k-tiles of 128 rows: p = 1..256, i = kt//2
WEIGHT_MAGNITUDE = float(np.sqrt(6.0 / (INPUT_DIM * (POLY_DEGREE + 1))))
TWO16 = 65536.0

_compiled = {}


def _build(loop_n=1):
    import concourse.tile as tile
    from concourse import bacc, mybir

    F32 = mybir.dt.float32
    F16 = mybir.dt.float16
    I32 = mybir.dt.int32
    I16 = mybir.dt.int16
    AF = mybir.ActivationFunctionType
    ALU = mybir.AluOpType

    nc = bacc.Bacc("TRN2", target_bir_lowering=False, debug=False)
    x_d = nc.dram_tensor("x", [BATCH, INPUT_DIM, S_SHARD], F32, kind="ExternalInput")
    w_d = nc.dram_tensor("w", [128, NKT * OUTPUT_DIM], F16, kind="ExternalInput")
    pc_d = nc.dram_tensor("pc", [128, 2], F32, kind="ExternalInput")
    bias_d = nc.dram_tensor("bias", [128, 2], F32, kind="ExternalInput")
    out_d = nc.dram_tensor("out", [BATCH, OUTPUT_DIM, S_SHARD], F32, kind="ExternalOutput")

    with tile.TileContext(nc) as tc:
        with (
            tc.tile_pool(name="const", bufs=1) as constp,
            tc.tile_pool(name="theta", bufs=1) as thp,
            tc.tile_pool(name="tmp", bufs=3) as tmpp,
            tc.tile_pool(name="bcast", bufs=3) as bcp,
            tc.tile_pool(name="yint", bufs=2) as yp,
            tc.tile_pool(name="tmat", bufs=3) as tp,
            tc.tile_pool(name="outs", bufs=4) as op,
            tc.tile_pool(name="psum", bufs=8, space="PSUM") as pp,
        ):
            w_t = constp.tile([128, NKT * OUTPUT_DIM], F16)
            nc.sync.dma_start(w_t[:], w_d[:])
            pc_t = constp.tile([128, 2], F32)
            nc.sync.dma_start(pc_t[:], pc_d[:])
            bias_t = constp.tile([128, 2], F32)
            nc.sync.dma_start(bias_t[:], bias_d[:])

            def body():
                # ---- theta stage: flat [96, 1024]; row = 48*b + 16*i + sc
                xt = thp.tile([96, 1024], F32)
                nc.sync.dma_start(xt[:], x_d[:].rearrange("b i (u c) -> (b i u) c", c=1024))
                sq = thp.tile([96, 1024], F32)
                nc.scalar.activation(sq[:], xt[:], AF.Square)
                r2 = thp.tile([96, 1024], F32)
                nc.scalar.activation(r2[:], sq[:], AF.Sqrt, bias=1.0, scale=-1.0)
                inv = thp.tile([96, 1024], F32)
                nc.vector.reciprocal(inv[:], r2[:])
                q = thp.tile([96, 1024], F32)
                nc.vector.tensor_mul(q[:], xt[:], inv[:])
                asn = thp.tile([96, 1024], F32)
                nc.scalar.activation(asn[:], q[:], AF.Arctan)
                # th16 = int16((pi/2 - asn) * 2^17/(2pi)) = int16(32768 - asn*2^17/2pi)
                th16 = thp.tile([96, 1024], I16)
                nc.vector.tensor_scalar(
                    th16[:], asn[:], float(-2.0 * TWO16 / (2 * np.pi)), 32768.0,
                    ALU.mult, ALU.add,
                )
                # repack to one row per (b, chunk): [32, 3*1024] i16, cols =
                # i-sections; 6 plain partition-slice DMAs
                th32 = thp.tile([32, INPUT_DIM * SC], I16)
                for b in range(BATCH):
                    for i in range(INPUT_DIM):
                        nc.sync.dma_start(
                            th32[16 * b:16 * (b + 1), i * SC:(i + 1) * SC],
                            th16[48 * b + 16 * i: 48 * b + 16 * i + 16, :])

                # ---- main loops, software-pipelined: chunk n computes while
                # chunk n-1 evacuates PSUM and DMAs out (keeps each engine's
                # FIFO stream stall-free)
                def evac_and_store(prev):
                    pb, psc, pps = prev
                    for m in range(2):
                        ob = op.tile([128, SC], F32)
                        for half in range(2):
                            sl = slice(half * 512, (half + 1) * 512)
                            if m == 0:
                                nc.vector.tensor_scalar(
                                    ob[:, sl], pps[m][half][:],
                                    bias_t[:, m:m + 1], None, ALU.add)
                            else:
                                nc.scalar.activation(
                                    ob[:, sl], pps[m][half][:], AF.Identity,
                                    bias=bias_t[:, m:m + 1])
                        dst = out_d[pb, m * 128:(m + 1) * 128,
                                    psc * SC:(psc + 1) * SC]
                        if m == 0:
                            nc.sync.dma_start(dst, ob[:])
                        else:
                            nc.scalar.dma_start(dst, ob[:])

                prev = None
                for b in range(BATCH):
                    for sc in range(NSC):
                        row = 16 * b + sc
                        # partition_broadcast needs a partition-0 source
                        tmp = tmpp.tile([1, INPUT_DIM * SC], I16)
                        nc.sync.dma_start(tmp[:], th32[row:row + 1, :])
                        # broadcast viewed as int32 pairs: gpsimd cost is
                        # per-column, so 1536 i32 cols beat 3072 i16 cols 2x
                        th_bc = bcp.tile([128, INPUT_DIM * SC // 2], I32)
                        nc.gpsimd.partition_broadcast(
                            th_bc[:], tmp[0:1, :].bitcast(I32))
                        th_bc16 = th_bc[:].bitcast(I16)

                        # k-tile order kt = parity*3 + i; p = parity*128+r+1,
                        # so one tensor_scalar spans all 3 i-sections per parity
                        y32 = yp.tile([128, NKT * SC], I32)
                        for par in range(2):
                            nc.vector.tensor_scalar(
                                y32[:, par * 3 * SC:(par + 1) * 3 * SC],
                                th_bc16,
                                pc_t[:, par:par + 1], 0.25 * TWO16, ALU.mult, ALU.add,
                            )
                        tm = tp.tile([128, NKT * SC], F16)
                        yv = y32[:].bitcast(I16).rearrange(
                            "p (n two) -> p n two", two=2)[:, :, 0]
                        nc.scalar.activation(tm[:], yv, AF.Sin,
                                             scale=float(2 * np.pi / TWO16))

                        if prev is not None:
                            evac_and_store(prev)

                        pss = []
                        for m in range(2):
                            phs = []
                            for half in range(2):
                                ps = pp.tile([128, 512], F32)
                                for kt in range(NKT):
                                    nc.tensor.matmul(
                                        ps[:],
                                        w_t[:, kt * OUTPUT_DIM + m * 128: kt * OUTPUT_DIM + m * 128 + 128],
                                        tm[:, kt * SC + half * 512: kt * SC + half * 512 + 512],
                                        start=(kt == 0), stop=(kt == NKT - 1),
                                    )
                                phs.append(ps)
                            pss.append(phs)
                        prev = (b, sc, pss)
                evac_and_store(prev)

            if loop_n == 1:
                body()
            else:
                with tc.For_i(0, loop_n, 1,
                              hint_engines=(mybir.EngineType.PE,),
                              staggered_reset=True):
                    body()
    nc.compile()
    return nc


def _host_prep(coefficients):
    w = (np.asarray(coefficients, dtype=np.float64) * WEIGHT_MAGNITUDE).astype(np.float32)
    # k-tile order kt = parity*3 + i:  wk[r, kt*256+o] = w[o, kt%3, (kt//3)*128+r+1]
    wk = np.empty((128, NKT * OUTPUT_DIM), np.float32)
    for kt in range(NKT):
        i = kt % 3
        p0 = (kt // 3) * 128 + 1
        wk[:, kt * OUTPUT_DIM:(kt + 1) * OUTPUT_DIM] = w[:, i, p0:p0 + 128].T
    r = np.arange(128)
    pc = np.empty((128, 2), np.float32)
    for par in range(2):
        pc[:, par] = (par * 128 + r + 1) * 0.5
    # bias[o', m] = sum_i w[m*128+o', i, 0]
    bias = np.ascontiguousarray(w[:, :, 0].sum(axis=1).reshape(2, 128).T.astype(np.float32))
    return wk.astype(np.float16), pc, bias


def _get_nc(loop_n=1):
    key = ("nc", loop_n)
    if key not in _compiled:
        _compiled[key] = _build(loop_n)
    return _compiled[key]


def _build_callable(nc, n_cores=N_CORES):
    """jit(shard_map(bass_exec)) over the first n_cores devices, mirroring
    run_bass_via_pjrt's lowering; inputs must be device_put with the
    returned sharding (axis 0 = per-core concat)."""
    import jax
    from jax.sharding import Mesh, PartitionSpec, NamedSharding
    from jax.experimental.shard_map import shard_map
    from concourse import mybir
    from concourse.bass2jax import (
        _bass_exec_p, install_neuronx_cc_hook, partition_id_tensor)

    install_neuronx_cc_hook()
    partition_name = nc.partition_id_tensor.name if nc.partition_id_tensor else None

    in_names, out_names, out_avals = [], [], []
    for alloc in nc.m.functions[0].allocations:
        if not isinstance(alloc, mybir.MemoryLocationSet):
            continue
        name = alloc.memorylocations[0].name
        if alloc.kind == "ExternalInput":
            if name != partition_name:
                in_names.append(name)
        elif alloc.kind == "ExternalOutput":
            out_names.append(name)
            out_avals.append(jax.core.ShapedArray(
                tuple(alloc.tensor_shape), mybir.dt.np(alloc.dtype)))
    n_params = len(in_names)
    n_outs = len(out_names)
    all_in_names = in_names + out_names
    if partition_name is not None:
        all_in_names.append(partition_name)

    def _body(*args):
        operands = list(args)
        if partition_name is not None:
            operands.append(partition_id_tensor())
        outs = _bass_exec_p.bind(
            *operands,
            out_avals=tuple(out_avals),
            in_names=tuple(all_in_names),
            out_names=tuple(out_names),
            lowering_input_output_aliases=(),
            sim_require_finite=True,
            sim_require_nnan=True,
            nc=nc,
        )
        return tuple(outs)

    devices = jax.devices()[:n_cores]
    mesh = Mesh(np.asarray(devices), ("core",))
    fn = jax.jit(shard_map(
        _body, mesh=mesh,
        in_specs=(PartitionSpec("core"),) * (n_params + n_outs),
        out_specs=(PartitionSpec("core"),) * n_outs, check_rep=False))
    return fn, NamedSharding(mesh, PartitionSpec("core")), in_names, out_avals


def _prep_globals(x, coefficients):
    """Per-core inputs concatenated along axis 0 (core-major)."""
    wk, pc, bias = _host_prep(coefficients)
    xg = np.ascontiguousarray(
        np.asarray(x, dtype=np.float32).reshape(BATCH, INPUT_DIM, N_CORES, S_SHARD)
        .transpose(2, 0, 1, 3).reshape(N_CORES * BATCH, INPUT_DIM, S_SHARD))
    wg = np.tile(wk, (N_CORES, 1))
    pcg = np.tile(pc, (N_CORES, 1))
    biasg = np.tile(bias, (N_CORES, 1))
    return {"x": xg, "w": wg, "pc": pcg, "bias": biasg}


def kernel(x, coefficients):
    from concourse import bass2jax

    nc = _get_nc()
    wk, pc, bias = _host_prep(coefficients)
    x = np.asarray(x, dtype=np.float32)
    in_maps = [
        {"x": np.ascontiguousarray(x[:, :, c * S_SHARD:(c + 1) * S_SHARD]),
         "w": wk, "pc": pc, "bias": bias}
        for c in range(N_CORES)
    ]
    results = bass2jax.run_bass_via_pjrt(nc, in_maps, n_cores=N_CORES)
    out = np.concatenate([results[c]["out"] for c in range(N_CORES)], axis=2)
    return np.ascontiguousarray(out.astype(np.float32))

